# revision 6
# baseline (speedup 1.0000x reference)
"""Trainium2 Bass kernel for nn_KnotEntangle (K=1024, SAMPLES=4096, 8 cores).

Math: the FFT collapses — signal[:,0] = rowsum(smear) and sum_sig = S*smear[:,0].
The [K,K]@[K,S] contraction reduces to result = w @ smear with
w = coef1 + coef2 + c3 built from row/col reductions of the mix matrix
(rank-1 corr => PE-computable deltas). Each gaussian is evaluated as
(sqrt(pi)/2)*Derivative_Erf(u), u = min(max(d*(-a), d*b), CLAMP) via one
custom DVE op; deltas come from rank-2 PE matmuls.

Sharding: knots split 128/core across 8 cores; [K] summaries AllGathered;
[2,4096] partial (result,env) rows AllReduced; output = env*result.

Host path: the jitted shard_map callable is built once and cached —
re-jitting per call (what run_bass_kernel_spmd does under axon) costs
~150ms/call in retrace+XLA. Outputs are fetched from shard 0 only, the
zero-donation convention is dropped (out is fully written on device),
and the executable is compiled with bass_effect suppressed so dispatch
takes jax's C++ fast path.
"""
import numpy as np

import jax
from jax.sharding import Mesh, PartitionSpec

import inspect as _inspect

try:
    from jax import shard_map as _shard_map  # jax >= 0.8
except ImportError:
    from jax.experimental.shard_map import shard_map as _shard_map

_SM_NOREP_KW = ("check_vma"
                if "check_vma" in _inspect.signature(_shard_map).parameters
                else "check_rep")


def shard_map(f, **kw):
    return _shard_map(f, **{_SM_NOREP_KW: False, **kw})

import concourse.bacc as bacc
import concourse.tile as tile
import concourse.mybir as mybir
from concourse import bass2jax

import concourse.dve_ops as dve_ops_mod
from concourse.dve_ops import DveOp, TENSOR_TENSOR_REDUCE
from concourse.dve_spec import Spec, Src0, C0, C1, C2, maxx, minn, lower as dve_lower
from concourse.dve_uop import DveOpSpec

K = 1024
SAMPLES = 4096
S = SAMPLES - 1           # 4095
M = 8                     # cores
KL = K // M               # 128 knots per core
SQ = float(np.sqrt(np.pi) / 2.0)
CLAMP = 30.0
CH = [(512 * i, 512) for i in range(7)] + [(3584, 511)]   # s-chunks


def _selmax_ref(in0, in1, s0, s1, imm2):
    return np.minimum(np.maximum(in0 * s0, in0 * s1), imm2).astype(np.float32)


def _make_selmax_op():
    name = "KNOT_SELMAX"
    if name in dve_ops_mod._SUB_OPCODE_FOR_NAME:
        return next(op for op in dve_ops_mod.OPS if op.name == name)
    spec = Spec(body=minn(maxx(Src0 * C0, Src0 * C1), C2), reference=_selmax_ref)
    row = dve_ops_mod._CUSTOM_DVE_ROW_BASE + len(dve_ops_mod.OPS)
    assert row < 0x20
    dve_ops_mod._SUB_OPCODE_FOR_NAME[name] = row
    shas = {}
    for ver in ("v3", "v4"):
        uops = dve_lower(spec, ver=ver)
        shas[ver] = DveOpSpec(name=name, opcode=row, uops=uops, rd1_en=False).sha(ver)
    op = DveOp(name, spec, subdim=False, uops_sha=shas)
    dve_ops_mod.OPS.append(op)
    dve_ops_mod.CUSTOM_DVE_SPECS[name] = spec
    return op


_CACHE = {}


def _build():
    if "nc" in _CACHE:
        return _CACHE["nc"]
    SELMAX = _make_selmax_op()
    nc = bacc.Bacc(None, target_bir_lowering=False, num_devices=M)
    f32 = mybir.dt.float32
    AF = mybir.ActivationFunctionType

    def din(name, shape):
        return nc.dram_tensor(name, shape, f32, kind="ExternalInput")

    smear_lhsT_d = din("smear_lhsT", [2, KL])
    env_lhsT_d = din("env_lhsT", [2, KL])
    nar_d = din("nar", [KL]);    br_d = din("brr", [KL])
    nae_d = din("nae", [KL]);    be_d = din("bee", [KL])
    naentL_d = din("naentL", [KL]); bentL_d = din("bentL", [KL])
    emloc_d = din("emloc", [KL]); nemloc_d = din("nemloc", [KL])
    naent8_d = din("naent8", [KL, M]); bent8_d = din("bent8", [KL, M])
    cosp8_d = din("cosp8", [KL, M])
    cospL_d = din("cospL", [KL]); sinpL_d = din("sinpL", [KL])
    nem_full_d = din("nem_full", [K])
    basis_d = din("basis", [2, S])

    out_d = nc.dram_tensor("out", [S], f32, kind="ExternalOutput")

    agin = nc.dram_tensor("agin", [2 * KL], f32, kind="Internal")
    agout = nc.dram_tensor("agout", [M * 2 * KL], f32, kind="Internal",
                           addr_space="Shared")
    arin = nc.dram_tensor("arin", [2, SAMPLES], f32, kind="Internal")
    arout = nc.dram_tensor("arout", [2, SAMPLES], f32, kind="Internal",
                           addr_space="Shared")
    coefd = nc.dram_tensor("coefd", [2 * KL], f32, kind="Internal")
    rg = [list(range(M))]

    with tile.TileContext(nc) as tc:
        with tc.tile_pool(name="big", bufs=1) as big, \
             tc.tile_pool(name="sml", bufs=1) as sml, \
             tc.tile_pool(name="u", bufs=3) as upool, \
             tc.tile_pool(name="acc", bufs=8) as accp, \
             tc.tile_pool(name="pd", bufs=3, space="PSUM") as pd, \
             tc.tile_pool(name="pr", bufs=1, space="PSUM") as pr, \
             tc.tile_pool(name="pc", bufs=1, space="PSUM") as pc:

            # ---- load constants ----
            basis = sml.tile([2, S], f32)
            nc.sync.dma_start(basis[:], basis_d[:])
            smear_lhsT = sml.tile([2, KL], f32)
            nc.sync.dma_start(smear_lhsT[:], smear_lhsT_d[:])
            env_lhsT = sml.tile([2, KL], f32)
            nc.sync.dma_start(env_lhsT[:], env_lhsT_d[:])

            def col(d):
                t = sml.tile([KL, 1], f32, tag=f"col_{d.name}")
                nc.sync.dma_start(t[:], d[:, None])
                return t

            nar = col(nar_d); br = col(br_d)
            nae = col(nae_d); be = col(be_d)
            naentL = col(naentL_d); bentL = col(bentL_d)
            emloc = col(emloc_d)
            cospL = col(cospL_d); sinpL = col(sinpL_d)
            naent8 = sml.tile([KL, M], f32)
            nc.sync.dma_start(naent8[:], naent8_d[:])
            bent8 = sml.tile([KL, M], f32)
            nc.sync.dma_start(bent8[:], bent8_d[:])
            cosp8 = sml.tile([KL, M], f32)
            nc.sync.dma_start(cosp8[:], cosp8_d[:])

            zero_col = sml.tile([KL, 1], f32)
            nc.vector.memset(zero_col[:], 0.0)
            ones_col = sml.tile([KL, 1], f32)
            nc.vector.memset(ones_col[:], 1.0)

            SM = big.tile([KL, S], f32)
            GA = big.tile([KL, S], f32)
            MXa = big.tile([KL, K], f32)
            MXb = big.tile([KL, K], f32)
            ssr_bc = big.tile([KL, K], f32)

            # ---- phase 2: smear ----
            acc8 = accp.tile([KL, M], f32, tag="acc8")
            for ci, (c0, n) in enumerate(CH):
                dl = pd.tile([KL, 512], f32, tag="delta")
                nc.tensor.matmul(dl[:, 0:n], smear_lhsT[:], basis[:, c0:c0 + n],
                                 start=True, stop=True)
                ut = upool.tile([KL, 512], f32, tag="u")
                nc.vector._custom_dve(SELMAX, out=ut[:, 0:n], in0=dl[:, 0:n],
                                      s0=nar[:], s1=br[:], imm2=CLAMP)
                nc.scalar.activation(SM[:, c0:c0 + n], ut[:, 0:n], AF.Derivative_Erf,
                                     bias=zero_col[:], accum_out=acc8[:, ci:ci + 1])
            ssr_p = sml.tile([KL, 1], f32)
            nc.vector.tensor_scalar_mul(ssr_p[:], SM[:, 0:1], float(S) * SQ)
            sig_sum = sml.tile([KL, 1], f32)
            nc.vector.reduce_sum(sig_sum[:], acc8[:], axis=mybir.AxisListType.X)
            sig0p = sml.tile([KL, 1], f32)
            nc.vector.tensor_scalar_mul(sig0p[:], sig_sum[:],
                                        float(SQ / np.sqrt(S)))

            # ---- phase 3: AllGather [sig0p | ssr] ----
            nc.sync.dma_start(agin[0:KL, None], sig0p[:])
            nc.sync.dma_start(agin[KL:2 * KL, None], ssr_p[:])
            nc.gpsimd.collective_compute(
                "AllGather", mybir.AluOpType.bypass, replica_groups=rg,
                ins=[agin[:]], outs=[agout[:]])

            # ---- phase 4: env (overlaps AG) ----
            for (c0, n) in CH:
                dl = pd.tile([KL, 512], f32, tag="delta")
                nc.tensor.matmul(dl[:, 0:n], env_lhsT[:], basis[:, c0:c0 + n],
                                 start=True, stop=True)
                ut = upool.tile([KL, 512], f32, tag="u")
                nc.vector._custom_dve(SELMAX, out=ut[:, 0:n], in0=dl[:, 0:n],
                                      s0=nae[:], s1=be[:], imm2=CLAMP)
                nc.scalar.activation(GA[:, c0:c0 + n], ut[:, 0:n], AF.Derivative_Erf,
                                     bias=zero_col[:])
            # env reduction: two [1,2048] psum halves sharing one pr slot
            env_row = sml.tile([1, S], f32)
            for h in range(2):
                red = pr.tile([1, 2048], f32, tag="red")
                base = 2048 * h
                nv = 2048 if h == 0 else S - 2048
                for (c0, n) in CH[4 * h:4 * h + 4]:
                    nc.tensor.matmul(red[0:1, c0 - base:c0 - base + n], ones_col[:],
                                     GA[:, c0:c0 + n], start=True, stop=True)
                nc.scalar.copy(env_row[0:1, base:base + nv], red[0:1, 0:nv])
            nc.sync.dma_start(arin[1, 0:S][None, :], env_row[:])

            # ---- phase 5: post-AG assembly ----
            rhs_b = sml.tile([2, K], f32)
            mixa_lhsT = sml.tile([2, K], f32)
            ssr8 = sml.tile([KL, M], f32)
            for r in range(M):
                nc.sync.dma_start(rhs_b[0:1, KL * r:KL * (r + 1)],
                                  agout[2 * KL * r:2 * KL * r + KL][None, :])
                nc.sync.dma_start(mixa_lhsT[0:1, KL * r:KL * (r + 1)],
                                  agout[2 * KL * r:2 * KL * r + KL][None, :])
                nc.sync.dma_start(ssr8[:, r:r + 1],
                                  agout[2 * KL * r + KL:2 * KL * (r + 1)][:, None])
                nc.sync.dma_start(
                    ssr_bc[:, KL * r:KL * (r + 1)],
                    agout[2 * KL * r + KL:2 * KL * (r + 1)][None, :]
                    .broadcast_to((KL, KL)))
            nc.sync.dma_start(rhs_b[1:2, :], basis_d[1, 0:K][None, :])
            nc.sync.dma_start(mixa_lhsT[1:2, :], nem_full_d[None, :])
            rhs_a = sml.tile([2, KL], f32)
            nc.sync.dma_start(rhs_a[0:1, :], agin[0:KL][None, :])
            nc.sync.dma_start(rhs_a[1:2, :], basis_d[1, 0:KL][None, :])
            mixb_lhsT = sml.tile([2, KL], f32)
            nc.sync.dma_start(mixb_lhsT[0:1, :], agin[0:KL][None, :])
            nc.sync.dma_start(mixb_lhsT[1:2, :], nemloc_d[None, :])
            cw8 = sml.tile([KL, M], f32)
            nc.vector.tensor_tensor(cw8[:], cosp8[:], ssr8[:],
                                    op=mybir.AluOpType.mult)
            wgt = sml.tile([KL, 2 * M], f32)
            nc.vector.memset(wgt[:], 1.0)
            for t in range(M):
                nc.vector.tensor_copy(wgt[:, 2 * t:2 * t + 1], cw8[:, t:t + 1])

            # ---- phase 6: mix block b (cc over global i) ----
            cch = []
            for ci, c0 in enumerate((0, 512)):
                dl = pd.tile([KL, 512], f32, tag="delta")
                nc.tensor.matmul(dl[:], mixb_lhsT[:], rhs_b[:, c0:c0 + 512],
                                 start=True, stop=True)
                ut = upool.tile([KL, 512], f32, tag="u")
                nc.vector._custom_dve(SELMAX, out=ut[:], in0=dl[:],
                                      s0=naentL[:], s1=bentL[:], imm2=CLAMP)
                nc.scalar.activation(MXb[:, c0:c0 + 512], ut[:], AF.Derivative_Erf,
                                     bias=zero_col[:])
                acc = accp.tile([KL, 1], f32, tag="cch")
                trash = upool.tile([KL, 512], f32, tag="u")
                nc.vector._custom_dve(TENSOR_TENSOR_REDUCE, out=trash[:],
                                      in0=MXb[:, c0:c0 + 512],
                                      in1=ssr_bc[:, c0:c0 + 512],
                                      s0=(0.0 if ci == 0 else cch[0][:]), s1=1.0,
                                      accum_out=acc[:])
                cch.append(acc)
            cchat = cch[1]

            # ---- phase 7: mix block a + coef reductions ----
            for t in range(M):
                dl = pd.tile([KL, KL], f32, tag="delta")
                nc.tensor.matmul(dl[:], mixa_lhsT[:, KL * t:KL * (t + 1)], rhs_a[:],
                                 start=True, stop=True)
                ut = upool.tile([KL, KL], f32, tag="u")
                nc.vector._custom_dve(SELMAX, out=ut[:], in0=dl[:],
                                      s0=naent8[:, t:t + 1], s1=bent8[:, t:t + 1],
                                      imm2=CLAMP)
                nc.scalar.activation(MXa[:, KL * t:KL * (t + 1)], ut[:],
                                     AF.Derivative_Erf, bias=zero_col[:])
            coef_ps = pc.tile([2, KL], f32)
            for t in range(M):
                nc.tensor.matmul(coef_ps[:], wgt[:, 2 * t:2 * t + 2],
                                 MXa[:, KL * t:KL * (t + 1)],
                                 start=(t == 0), stop=(t == M - 1))
            coef_sb = sml.tile([2, KL], f32)
            nc.scalar.copy(coef_sb[:], coef_ps[:])
            nc.sync.dma_start(coefd[:].rearrange("(a b) -> a b", a=2), coef_sb[:])
            coef_t = sml.tile([KL, 2], f32)
            nc.sync.dma_start(coef_t[:], coefd[:].rearrange("(two k) -> k two", two=2))

            # ---- phase 8: diag + w ----
            TT = nc.vector.tensor_tensor
            A = mybir.AluOpType
            dd = sml.tile([KL, 1], f32)
            TT(dd[:], sig0p[:], sig0p[:], op=A.mult)
            TT(dd[:], dd[:], emloc[:], op=A.subtract)
            udg = sml.tile([KL, 1], f32)
            nc.vector._custom_dve(SELMAX, out=udg[:], in0=dd[:],
                                  s0=naentL[:], s1=bentL[:], imm2=CLAMP)
            MD = sml.tile([KL, 1], f32)
            nc.scalar.activation(MD[:], udg[:], AF.Derivative_Erf, bias=zero_col[:])

            cwL = sml.tile([KL, 1], f32)
            TT(cwL[:], cospL[:], ssr_p[:], op=A.mult)
            t2 = sml.tile([KL, 1], f32)
            TT(t2[:], MD[:], cwL[:], op=A.mult)
            coef1 = sml.tile([KL, 1], f32)
            TT(coef1[:], coef_t[:, 0:1], t2[:], op=A.subtract)
            nc.vector.tensor_scalar_mul(coef1[:], coef1[:], SQ)
            rsnd = sml.tile([KL, 1], f32)
            TT(rsnd[:], coef_t[:, 1:2], MD[:], op=A.subtract)
            c3 = sml.tile([KL, 1], f32)
            nc.vector.tensor_scalar(c3[:], rsnd[:], -SQ, float(K - 1),
                                    op0=A.mult, op1=A.add)
            ccm = sml.tile([KL, 1], f32)
            TT(ccm[:], MD[:], ssr_p[:], op=A.mult)
            cc = sml.tile([KL, 1], f32)
            TT(cc[:], cchat[:], ccm[:], op=A.subtract)
            nc.vector.tensor_scalar_mul(cc[:], cc[:], SQ)
            coef2 = sml.tile([KL, 1], f32)
            TT(coef2[:], sinpL[:], cc[:], op=A.mult)
            wv = sml.tile([KL, 1], f32)
            TT(wv[:], coef1[:], coef2[:], op=A.add)
            TT(wv[:], wv[:], c3[:], op=A.add)
            wf = sml.tile([KL, 1], f32)
            nc.vector.tensor_scalar_mul(wf[:], wv[:], float(np.pi / 4.0))

            # ---- phase 9: result reduction ----
            res_row = sml.tile([1, S], f32)
            for h in range(2):
                red = pr.tile([1, 2048], f32, tag="red")
                base = 2048 * h
                nv = 2048 if h == 0 else S - 2048
                for (c0, n) in CH[4 * h:4 * h + 4]:
                    nc.tensor.matmul(red[0:1, c0 - base:c0 - base + n], wf[:],
                                     SM[:, c0:c0 + n], start=True, stop=True)
                nc.scalar.copy(res_row[0:1, base:base + nv], red[0:1, 0:nv])
            nc.sync.dma_start(arin[0, 0:S][None, :], res_row[:])

            # ---- phase 10: AllReduce ----
            nc.gpsimd.collective_compute(
                "AllReduce", A.add, replica_groups=rg,
                ins=[arin[:]], outs=[arout[:]])

            # ---- phase 11: final product ----
            res_t = sml.tile([KL, 32], f32)
            env_t = sml.tile([KL, 32], f32)
            nc.sync.dma_start(res_t[:], arout[0, :].rearrange("(p c) -> p c", c=32))
            nc.sync.dma_start(env_t[:], arout[1, :].rearrange("(p c) -> p c", c=32))
            out_t = sml.tile([KL, 32], f32)
            TT(out_t[:], res_t[:], env_t[:], op=A.mult)
            nc.sync.dma_start(out_d[0:4064].rearrange("(p c) -> p c", c=32),
                              out_t[0:127, :])
            nc.sync.dma_start(out_d[4064:S][None, :], out_t[127:128, 0:31])

    nc.compile()
    _CACHE["nc"] = nc
    return nc


def _concat_inputs(x, smear_window, knot_mean, knot_low, knot_high,
                   ent_mean, ent_low, ent_high, polarization):
    """Global (8*local_shape) input arrays, keyed by dram-tensor name.

    Cores take contiguous K-slices, so axis-0 concat of per-core [KL]
    slices is just the full [K] array; per-core-identical inputs tile."""
    lo = float(smear_window[0]); up = float(smear_window[1])
    x = np.asarray(x, np.float32)
    r2 = np.float32(1.0 / np.sqrt(2.0))
    km = np.asarray(knot_mean, np.float32)
    aent_f = np.exp(-np.asarray(ent_low, np.float32)) * r2
    bent_f = np.exp(-np.asarray(ent_high, np.float32)) * r2
    cos_f = np.cos(np.asarray(polarization, np.float32))
    sin_f = np.sin(np.asarray(polarization, np.float32))
    em_f = np.asarray(ent_mean, np.float32)

    def inter2(a, b):
        # concat of per-core [2,KL] stacks: rows a[L0],b[L0],a[L1],b[L1],...
        return np.stack([a.reshape(M, KL), b.reshape(M, KL)],
                        axis=1).reshape(2 * M, KL)

    def tile8(a):
        return np.tile(a, (M,) + (1,) * (a.ndim - 1)) if a.ndim > 1 \
            else np.tile(a, M)

    xs = np.float32((up - lo) / SAMPLES) * x
    xm = np.float32(1.0 - lo) * x - km
    es = np.float32(up + lo) * x
    eo = np.float32(-lo) * x
    basis = np.stack([np.arange(S, dtype=np.float32) / SAMPLES,
                      np.ones(S, dtype=np.float32)])
    return {
        "smear_lhsT": inter2(xs, xm),
        "env_lhsT": inter2(es, eo),
        "nar": -np.exp(-np.asarray(knot_low, np.float32)) * r2,
        "brr": np.exp(-np.asarray(knot_high, np.float32)) * r2,
        "nae": -np.exp(-np.float32(1.0 - lo) * x) * r2,
        "bee": np.exp(-np.float32(1.0 + up) * x) * r2,
        "naentL": -aent_f, "bentL": bent_f,
        "emloc": em_f, "nemloc": -em_f,
        "naent8": tile8(np.ascontiguousarray((-aent_f).reshape(M, KL).T)),
        "bent8": tile8(np.ascontiguousarray(bent_f.reshape(M, KL).T)),
        "cosp8": tile8(np.ascontiguousarray(cos_f.reshape(M, KL).T)),
        "cospL": cos_f, "sinpL": sin_f,
        "nem_full": tile8(-em_f),
        "basis": tile8(basis),
    }


def _get_fn(nc, concat_in):
    """Build (once) the cached fast-dispatch jitted executable."""
    if "fn" in _CACHE:
        return _CACHE["fn"]
    bass2jax.install_neuronx_cc_hook()
    partition_name = nc.partition_id_tensor.name if nc.partition_id_tensor else None
    in_names, out_names, out_avals = [], [], []
    for alloc in nc.m.functions[0].allocations:
        if not isinstance(alloc, mybir.MemoryLocationSet):
            continue
        name = alloc.memorylocations[0].name
        if alloc.kind == "ExternalInput":
            if name != partition_name:
                in_names.append(name)
        elif alloc.kind == "ExternalOutput":
            out_names.append(name)
            out_avals.append(jax.core.ShapedArray(tuple(alloc.tensor_shape),
                                                  mybir.dt.np(alloc.dtype)))
    names_all = in_names + ([partition_name] if partition_name else [])

    def _body(*args):
        operands = list(args)
        if partition_name:
            operands = operands + [bass2jax.partition_id_tensor()]
        return tuple(bass2jax._bass_exec_p.bind(
            *operands, out_avals=tuple(out_avals), in_names=tuple(names_all),
            out_names=tuple(out_names), lowering_input_output_aliases=(),
            sim_require_finite=True, sim_require_nnan=True, nc=nc))

    mesh = Mesh(np.asarray(jax.devices()[:M]), ("core",))
    smap = shard_map(_body, mesh=mesh,
                     in_specs=(PartitionSpec("core"),) * len(in_names),
                     out_specs=(PartitionSpec("core"),) * len(out_names))
    try:
        fn = bass2jax.fast_dispatch_compile(
            lambda: jax.jit(smap, keep_unused=True).lower(*concat_in).compile())
    except Exception:
        fn = jax.jit(smap, keep_unused=True)
    _CACHE["fn"] = fn
    _CACHE["in_names"] = in_names
    return fn


def kernel(x, smear_window, knot_mean, knot_low, knot_high,
           ent_mean, ent_low, ent_high, polarization):
    nc = _build()
    ins = _concat_inputs(x, smear_window, knot_mean, knot_low, knot_high,
                         ent_mean, ent_low, ent_high, polarization)
    in_names = _CACHE.get("in_names")
    if in_names is None:
        partition_name = (nc.partition_id_tensor.name
                          if nc.partition_id_tensor else None)
        in_names = [
            alloc.memorylocations[0].name
            for alloc in nc.m.functions[0].allocations
            if isinstance(alloc, mybir.MemoryLocationSet)
            and alloc.kind == "ExternalInput"
            and alloc.memorylocations[0].name != partition_name
        ]
    concat_in = [ins[name] for name in in_names]
    fn = _get_fn(nc, concat_in)
    out_arrs = fn(*concat_in)
    return np.asarray(out_arrs[0].addressable_shards[0].data).astype(np.float32)


# revision 7
# speedup vs baseline: 1.6014x; 1.6014x over previous
"""Trainium2 Bass kernel for nn_KnotEntangle (K=1024, SAMPLES=4096, 8 cores).

Math: the FFT collapses — signal[:,0] = rowsum(smear) and sum_sig = S*smear[:,0].
The [K,K]@[K,S] contraction reduces to result = w @ smear with
w = coef1 + coef2 + c3 built from row/col reductions of the mix matrix
(rank-1 corr => PE-computable deltas). Each gaussian is evaluated as
(sqrt(pi)/2)*Derivative_Erf(u), u = min(max(d*(-a), d*b), CLAMP) via one
custom DVE op; deltas come from rank-2 PE matmuls.

Sharding: knots split 128/core across 8 cores; [K] summaries AllGathered;
[2,4096] partial (result,env) rows AllReduced; output = env*result.

Host path (the measured wall-clock is dominated by the axon tunnel, not
device compute — TimelineSim puts the NEFF at ~135us/core):
- the jitted shard_map callable is built once and cached; re-jitting per
  call (what run_bass_kernel_spmd does under axon) costs ~150ms/call,
- all per-core inputs are packed into ONE flat f32 dram tensor so each
  call ships a single host->device buffer (17 separate buffers measured
  ~13ms slower per call),
- the [2,S] iota/ones basis is generated on device,
- compiled via fast_dispatch_compile (bass_effect suppressed, C++ path),
- output fetched from shard 0 only; no donated zero-output buffers
  (out is fully written on device).
"""
import inspect as _inspect

import numpy as np

import jax
from jax.sharding import Mesh, PartitionSpec

try:
    from jax import shard_map as _shard_map  # jax >= 0.8
except ImportError:
    from jax.experimental.shard_map import shard_map as _shard_map

_SM_NOREP_KW = ("check_vma"
                if "check_vma" in _inspect.signature(_shard_map).parameters
                else "check_rep")


def shard_map(f, **kw):
    return _shard_map(f, **{_SM_NOREP_KW: False, **kw})


import concourse.bacc as bacc
import concourse.tile as tile
import concourse.mybir as mybir
from concourse import bass2jax

import concourse.dve_ops as dve_ops_mod
from concourse.dve_ops import DveOp, TENSOR_TENSOR_REDUCE
from concourse.dve_spec import Spec, Src0, C0, C1, C2, maxx, minn, lower as dve_lower
from concourse.dve_uop import DveOpSpec

K = 1024
SAMPLES = 4096
S = SAMPLES - 1           # 4095
M = 8                     # cores
KL = K // M               # 128 knots per core
SQ = float(np.sqrt(np.pi) / 2.0)
CLAMP = 30.0
CH = [(512 * i, 512) for i in range(7)] + [(3584, 511)]   # s-chunks

# packed-input blob layout (f32 offsets, per core)
OFF = {}
_o = 0
for _name, _sz in [("smear_lhsT", 2 * KL), ("env_lhsT", 2 * KL),
                   ("nar", KL), ("brr", KL), ("nae", KL), ("bee", KL),
                   ("naentL", KL), ("bentL", KL), ("emloc", KL),
                   ("nemloc", KL), ("cospL", KL), ("sinpL", KL),
                   ("naent8", KL * M), ("bent8", KL * M), ("cosp8", KL * M),
                   ("nem_full", K)]:
    OFF[_name] = _o
    _o += _sz
BLOB = _o  # 5888


def _selmax_ref(in0, in1, s0, s1, imm2):
    return np.minimum(np.maximum(in0 * s0, in0 * s1), imm2).astype(np.float32)


def _make_selmax_op():
    name = "KNOT_SELMAX"
    if name in dve_ops_mod._SUB_OPCODE_FOR_NAME:
        return next(op for op in dve_ops_mod.OPS if op.name == name)
    spec = Spec(body=minn(maxx(Src0 * C0, Src0 * C1), C2), reference=_selmax_ref)
    row = dve_ops_mod._CUSTOM_DVE_ROW_BASE + len(dve_ops_mod.OPS)
    assert row < 0x20
    dve_ops_mod._SUB_OPCODE_FOR_NAME[name] = row
    shas = {}
    for ver in ("v3", "v4"):
        uops = dve_lower(spec, ver=ver)
        shas[ver] = DveOpSpec(name=name, opcode=row, uops=uops, rd1_en=False).sha(ver)
    op = DveOp(name, spec, subdim=False, uops_sha=shas)
    dve_ops_mod.OPS.append(op)
    dve_ops_mod.CUSTOM_DVE_SPECS[name] = spec
    return op


_CACHE = {}


def _build():
    if "nc" in _CACHE:
        return _CACHE["nc"]
    SELMAX = _make_selmax_op()
    nc = bacc.Bacc(None, target_bir_lowering=False, num_devices=M)
    f32 = mybir.dt.float32
    AF = mybir.ActivationFunctionType

    blobd = nc.dram_tensor("blob", [BLOB], f32, kind="ExternalInput")
    out_d = nc.dram_tensor("out", [S], f32, kind="ExternalOutput")

    agin = nc.dram_tensor("agin", [2 * KL], f32, kind="Internal")
    agout = nc.dram_tensor("agout", [M * 2 * KL], f32, kind="Internal",
                           addr_space="Shared")
    arin = nc.dram_tensor("arin", [2, SAMPLES], f32, kind="Internal")
    arout = nc.dram_tensor("arout", [2, SAMPLES], f32, kind="Internal",
                           addr_space="Shared")
    coefd = nc.dram_tensor("coefd", [2 * KL], f32, kind="Internal")
    rg = [list(range(M))]

    with tile.TileContext(nc) as tc:
        with tc.tile_pool(name="big", bufs=1) as big, \
             tc.tile_pool(name="sml", bufs=1) as sml, \
             tc.tile_pool(name="u", bufs=3) as upool, \
             tc.tile_pool(name="acc", bufs=8) as accp, \
             tc.tile_pool(name="pd", bufs=3, space="PSUM") as pd, \
             tc.tile_pool(name="pr", bufs=1, space="PSUM") as pr, \
             tc.tile_pool(name="pc", bufs=1, space="PSUM") as pc:

            # ---- constants: basis on device (row0 = iota/SAMPLES, row1 = 1) ----
            basis = sml.tile([2, S], f32)
            nc.vector.memset(basis[:], 1.0)
            nc.gpsimd.iota(basis[0:1, :], pattern=[[1, S]], base=0,
                           channel_multiplier=0,
                           allow_small_or_imprecise_dtypes=True)
            nc.vector.tensor_scalar_mul(basis[0:1, :], basis[0:1, :],
                                        1.0 / SAMPLES)

            smear_lhsT = sml.tile([2, KL], f32)
            nc.sync.dma_start(smear_lhsT[:],
                              blobd[OFF["smear_lhsT"]:OFF["smear_lhsT"] + 2 * KL]
                              .rearrange("(a b) -> a b", b=KL))
            env_lhsT = sml.tile([2, KL], f32)
            nc.sync.dma_start(env_lhsT[:],
                              blobd[OFF["env_lhsT"]:OFF["env_lhsT"] + 2 * KL]
                              .rearrange("(a b) -> a b", b=KL))

            def col(name):
                t = sml.tile([KL, 1], f32, tag=f"col_{name}")
                nc.sync.dma_start(t[:], blobd[OFF[name]:OFF[name] + KL][:, None])
                return t

            nar = col("nar"); br = col("brr")
            nae = col("nae"); be = col("bee")
            naentL = col("naentL"); bentL = col("bentL")
            emloc = col("emloc")
            cospL = col("cospL"); sinpL = col("sinpL")

            def mat8(name):
                t = sml.tile([KL, M], f32, tag=f"m8_{name}")
                nc.sync.dma_start(t[:], blobd[OFF[name]:OFF[name] + KL * M]
                                  .rearrange("(k m) -> k m", m=M))
                return t

            naent8 = mat8("naent8"); bent8 = mat8("bent8"); cosp8 = mat8("cosp8")

            zero_col = sml.tile([KL, 1], f32)
            nc.vector.memset(zero_col[:], 0.0)
            ones_col = sml.tile([KL, 1], f32)
            nc.vector.memset(ones_col[:], 1.0)

            SM = big.tile([KL, S], f32)
            GA = big.tile([KL, S], f32)
            MXa = big.tile([KL, K], f32)
            MXb = big.tile([KL, K], f32)
            ssr_bc = big.tile([KL, K], f32)

            # ---- phase 2: smear ----
            acc8 = accp.tile([KL, M], f32, tag="acc8")
            for ci, (c0, n) in enumerate(CH):
                dl = pd.tile([KL, 512], f32, tag="delta")
                nc.tensor.matmul(dl[:, 0:n], smear_lhsT[:], basis[:, c0:c0 + n],
                                 start=True, stop=True)
                ut = upool.tile([KL, 512], f32, tag="u")
                nc.vector._custom_dve(SELMAX, out=ut[:, 0:n], in0=dl[:, 0:n],
                                      s0=nar[:], s1=br[:], imm2=CLAMP)
                nc.scalar.activation(SM[:, c0:c0 + n], ut[:, 0:n],
                                     AF.Derivative_Erf,
                                     bias=zero_col[:], accum_out=acc8[:, ci:ci + 1])
            ssr_p = sml.tile([KL, 1], f32)
            nc.vector.tensor_scalar_mul(ssr_p[:], SM[:, 0:1], float(S) * SQ)
            sig_sum = sml.tile([KL, 1], f32)
            nc.vector.reduce_sum(sig_sum[:], acc8[:], axis=mybir.AxisListType.X)
            sig0p = sml.tile([KL, 1], f32)
            nc.vector.tensor_scalar_mul(sig0p[:], sig_sum[:],
                                        float(SQ / np.sqrt(S)))

            # ---- phase 3: AllGather [sig0p | ssr] ----
            nc.sync.dma_start(agin[0:KL, None], sig0p[:])
            nc.sync.dma_start(agin[KL:2 * KL, None], ssr_p[:])
            nc.gpsimd.collective_compute(
                "AllGather", mybir.AluOpType.bypass, replica_groups=rg,
                ins=[agin[:]], outs=[agout[:]])

            # ---- phase 4: env (overlaps AG) ----
            for (c0, n) in CH:
                dl = pd.tile([KL, 512], f32, tag="delta")
                nc.tensor.matmul(dl[:, 0:n], env_lhsT[:], basis[:, c0:c0 + n],
                                 start=True, stop=True)
                ut = upool.tile([KL, 512], f32, tag="u")
                nc.vector._custom_dve(SELMAX, out=ut[:, 0:n], in0=dl[:, 0:n],
                                      s0=nae[:], s1=be[:], imm2=CLAMP)
                nc.scalar.activation(GA[:, c0:c0 + n], ut[:, 0:n],
                                     AF.Derivative_Erf, bias=zero_col[:])
            env_row = sml.tile([1, S], f32)
            for h in range(2):
                red = pr.tile([1, 2048], f32, tag="red")
                base = 2048 * h
                nv = 2048 if h == 0 else S - 2048
                for (c0, n) in CH[4 * h:4 * h + 4]:
                    nc.tensor.matmul(red[0:1, c0 - base:c0 - base + n],
                                     ones_col[:], GA[:, c0:c0 + n],
                                     start=True, stop=True)
                nc.scalar.copy(env_row[0:1, base:base + nv], red[0:1, 0:nv])
            nc.sync.dma_start(arin[1, 0:S][None, :], env_row[:])

            # ---- phase 5: post-AG assembly ----
            rhs_b = sml.tile([2, K], f32)
            nc.vector.memset(rhs_b[:], 1.0)
            mixa_lhsT = sml.tile([2, K], f32)
            ssr8 = sml.tile([KL, M], f32)
            for r in range(M):
                nc.sync.dma_start(rhs_b[0:1, KL * r:KL * (r + 1)],
                                  agout[2 * KL * r:2 * KL * r + KL][None, :])
                nc.sync.dma_start(mixa_lhsT[0:1, KL * r:KL * (r + 1)],
                                  agout[2 * KL * r:2 * KL * r + KL][None, :])
                nc.sync.dma_start(ssr8[:, r:r + 1],
                                  agout[2 * KL * r + KL:2 * KL * (r + 1)][:, None])
                nc.sync.dma_start(
                    ssr_bc[:, KL * r:KL * (r + 1)],
                    agout[2 * KL * r + KL:2 * KL * (r + 1)][None, :]
                    .broadcast_to((KL, KL)))
            nc.sync.dma_start(mixa_lhsT[1:2, :],
                              blobd[OFF["nem_full"]:OFF["nem_full"] + K][None, :])
            rhs_a = sml.tile([2, KL], f32)
            nc.vector.memset(rhs_a[:], 1.0)
            nc.sync.dma_start(rhs_a[0:1, :], agin[0:KL][None, :])
            mixb_lhsT = sml.tile([2, KL], f32)
            nc.sync.dma_start(mixb_lhsT[0:1, :], agin[0:KL][None, :])
            nc.sync.dma_start(mixb_lhsT[1:2, :],
                              blobd[OFF["nemloc"]:OFF["nemloc"] + KL][None, :])
            cw8 = sml.tile([KL, M], f32)
            nc.vector.tensor_tensor(cw8[:], cosp8[:], ssr8[:],
                                    op=mybir.AluOpType.mult)
            wgt = sml.tile([KL, 2 * M], f32)
            nc.vector.memset(wgt[:], 1.0)
            for t in range(M):
                nc.vector.tensor_copy(wgt[:, 2 * t:2 * t + 1], cw8[:, t:t + 1])

            # ---- phase 6: mix block b (cc over global i) ----
            cch = []
            for ci, c0 in enumerate((0, 512)):
                dl = pd.tile([KL, 512], f32, tag="delta")
                nc.tensor.matmul(dl[:], mixb_lhsT[:], rhs_b[:, c0:c0 + 512],
                                 start=True, stop=True)
                ut = upool.tile([KL, 512], f32, tag="u")
                nc.vector._custom_dve(SELMAX, out=ut[:], in0=dl[:],
                                      s0=naentL[:], s1=bentL[:], imm2=CLAMP)
                nc.scalar.activation(MXb[:, c0:c0 + 512], ut[:],
                                     AF.Derivative_Erf, bias=zero_col[:])
                acc = accp.tile([KL, 1], f32, tag="cch")
                trash = upool.tile([KL, 512], f32, tag="u")
                nc.vector._custom_dve(TENSOR_TENSOR_REDUCE, out=trash[:],
                                      in0=MXb[:, c0:c0 + 512],
                                      in1=ssr_bc[:, c0:c0 + 512],
                                      s0=(0.0 if ci == 0 else cch[0][:]), s1=1.0,
                                      accum_out=acc[:])
                cch.append(acc)
            cchat = cch[1]

            # ---- phase 7: mix block a + coef reductions ----
            for t in range(M):
                dl = pd.tile([KL, KL], f32, tag="delta")
                nc.tensor.matmul(dl[:], mixa_lhsT[:, KL * t:KL * (t + 1)],
                                 rhs_a[:], start=True, stop=True)
                ut = upool.tile([KL, KL], f32, tag="u")
                nc.vector._custom_dve(SELMAX, out=ut[:], in0=dl[:],
                                      s0=naent8[:, t:t + 1], s1=bent8[:, t:t + 1],
                                      imm2=CLAMP)
                nc.scalar.activation(MXa[:, KL * t:KL * (t + 1)], ut[:],
                                     AF.Derivative_Erf, bias=zero_col[:])
            coef_ps = pc.tile([2, KL], f32)
            for t in range(M):
                nc.tensor.matmul(coef_ps[:], wgt[:, 2 * t:2 * t + 2],
                                 MXa[:, KL * t:KL * (t + 1)],
                                 start=(t == 0), stop=(t == M - 1))
            coef_sb = sml.tile([2, KL], f32)
            nc.scalar.copy(coef_sb[:], coef_ps[:])
            nc.sync.dma_start(coefd[:].rearrange("(a b) -> a b", a=2), coef_sb[:])
            coef_t = sml.tile([KL, 2], f32)
            nc.sync.dma_start(coef_t[:],
                              coefd[:].rearrange("(two k) -> k two", two=2))

            # ---- phase 8: diag + w ----
            TT = nc.vector.tensor_tensor
            A = mybir.AluOpType
            dd = sml.tile([KL, 1], f32)
            TT(dd[:], sig0p[:], sig0p[:], op=A.mult)
            TT(dd[:], dd[:], emloc[:], op=A.subtract)
            udg = sml.tile([KL, 1], f32)
            nc.vector._custom_dve(SELMAX, out=udg[:], in0=dd[:],
                                  s0=naentL[:], s1=bentL[:], imm2=CLAMP)
            MD = sml.tile([KL, 1], f32)
            nc.scalar.activation(MD[:], udg[:], AF.Derivative_Erf,
                                 bias=zero_col[:])

            cwL = sml.tile([KL, 1], f32)
            TT(cwL[:], cospL[:], ssr_p[:], op=A.mult)
            t2 = sml.tile([KL, 1], f32)
            TT(t2[:], MD[:], cwL[:], op=A.mult)
            coef1 = sml.tile([KL, 1], f32)
            TT(coef1[:], coef_t[:, 0:1], t2[:], op=A.subtract)
            nc.vector.tensor_scalar_mul(coef1[:], coef1[:], SQ)
            rsnd = sml.tile([KL, 1], f32)
            TT(rsnd[:], coef_t[:, 1:2], MD[:], op=A.subtract)
            c3 = sml.tile([KL, 1], f32)
            nc.vector.tensor_scalar(c3[:], rsnd[:], -SQ, float(K - 1),
                                    op0=A.mult, op1=A.add)
            ccm = sml.tile([KL, 1], f32)
            TT(ccm[:], MD[:], ssr_p[:], op=A.mult)
            cc = sml.tile([KL, 1], f32)
            TT(cc[:], cchat[:], ccm[:], op=A.subtract)
            nc.vector.tensor_scalar_mul(cc[:], cc[:], SQ)
            coef2 = sml.tile([KL, 1], f32)
            TT(coef2[:], sinpL[:], cc[:], op=A.mult)
            wv = sml.tile([KL, 1], f32)
            TT(wv[:], coef1[:], coef2[:], op=A.add)
            TT(wv[:], wv[:], c3[:], op=A.add)
            wf = sml.tile([KL, 1], f32)
            nc.vector.tensor_scalar_mul(wf[:], wv[:], float(np.pi / 4.0))

            # ---- phase 9: result reduction ----
            res_row = sml.tile([1, S], f32)
            for h in range(2):
                red = pr.tile([1, 2048], f32, tag="red")
                base = 2048 * h
                nv = 2048 if h == 0 else S - 2048
                for (c0, n) in CH[4 * h:4 * h + 4]:
                    nc.tensor.matmul(red[0:1, c0 - base:c0 - base + n], wf[:],
                                     SM[:, c0:c0 + n], start=True, stop=True)
                nc.scalar.copy(res_row[0:1, base:base + nv], red[0:1, 0:nv])
            nc.sync.dma_start(arin[0, 0:S][None, :], res_row[:])

            # ---- phase 10: AllReduce ----
            nc.gpsimd.collective_compute(
                "AllReduce", A.add, replica_groups=rg,
                ins=[arin[:]], outs=[arout[:]])

            # ---- phase 11: final product ----
            res_t = sml.tile([KL, 32], f32)
            env_t = sml.tile([KL, 32], f32)
            nc.sync.dma_start(res_t[:], arout[0, :].rearrange("(p c) -> p c", c=32))
            nc.sync.dma_start(env_t[:], arout[1, :].rearrange("(p c) -> p c", c=32))
            out_t = sml.tile([KL, 32], f32)
            TT(out_t[:], res_t[:], env_t[:], op=A.mult)
            nc.sync.dma_start(out_d[0:4064].rearrange("(p c) -> p c", c=32),
                              out_t[0:127, :])
            nc.sync.dma_start(out_d[4064:S][None, :], out_t[127:128, 0:31])

    nc.compile()
    _CACHE["nc"] = nc
    return nc


def _pack_blob(x, smear_window, knot_mean, knot_low, knot_high,
               ent_mean, ent_low, ent_high, polarization):
    """Pack all per-core derived inputs into one [M*BLOB] f32 array.

    Cores take contiguous K-slices, so a [K] per-knot array reshaped to
    [M, KL] gives row c = core c's slice; per-core-identical blocks are
    broadcast across the M rows."""
    lo = float(smear_window[0]); up = float(smear_window[1])
    x = np.asarray(x, np.float32)
    r2 = np.float32(1.0 / np.sqrt(2.0))
    km = np.asarray(knot_mean, np.float32)
    aent_f = np.exp(-np.asarray(ent_low, np.float32)) * r2
    bent_f = np.exp(-np.asarray(ent_high, np.float32)) * r2
    cos_f = np.cos(np.asarray(polarization, np.float32))
    sin_f = np.sin(np.asarray(polarization, np.float32))
    em_f = np.asarray(ent_mean, np.float32)
    xs = np.float32((up - lo) / SAMPLES) * x
    xm = np.float32(1.0 - lo) * x - km
    es = np.float32(up + lo) * x
    eo = np.float32(-lo) * x

    def cols(a):           # [K] -> [M, KL]
        return a.reshape(M, KL)

    parts = [
        np.hstack([cols(xs), cols(xm)]),                  # smear_lhsT
        np.hstack([cols(es), cols(eo)]),                  # env_lhsT
        cols(-np.exp(-np.asarray(knot_low, np.float32)) * r2),
        cols(np.exp(-np.asarray(knot_high, np.float32)) * r2),
        cols(-np.exp(-np.float32(1.0 - lo) * x) * r2),
        cols(np.exp(-np.float32(1.0 + up) * x) * r2),
        cols(-aent_f), cols(bent_f), cols(em_f), cols(-em_f),
        cols(cos_f), cols(sin_f),
        np.broadcast_to(np.ascontiguousarray(
            (-aent_f).reshape(M, KL).T).reshape(-1), (M, KL * M)),
        np.broadcast_to(np.ascontiguousarray(
            bent_f.reshape(M, KL).T).reshape(-1), (M, KL * M)),
        np.broadcast_to(np.ascontiguousarray(
            cos_f.reshape(M, KL).T).reshape(-1), (M, KL * M)),
        np.broadcast_to(-em_f, (M, K)),
    ]
    return np.ascontiguousarray(np.hstack(parts), dtype=np.float32).reshape(-1)


def _get_fn(nc, blob):
    """Build (once) the cached fast-dispatch jitted executable."""
    if "fn" in _CACHE:
        return _CACHE["fn"]
    bass2jax.install_neuronx_cc_hook()
    partition_name = nc.partition_id_tensor.name if nc.partition_id_tensor else None
    out_names, out_avals = [], []
    for alloc in nc.m.functions[0].allocations:
        if not isinstance(alloc, mybir.MemoryLocationSet):
            continue
        if alloc.kind == "ExternalOutput":
            out_names.append(alloc.memorylocations[0].name)
            out_avals.append(jax.core.ShapedArray(tuple(alloc.tensor_shape),
                                                  mybir.dt.np(alloc.dtype)))
    names_all = ["blob"] + ([partition_name] if partition_name else [])

    def _body(*args):
        operands = list(args)
        if partition_name:
            operands = operands + [bass2jax.partition_id_tensor()]
        return tuple(bass2jax._bass_exec_p.bind(
            *operands, out_avals=tuple(out_avals), in_names=tuple(names_all),
            out_names=tuple(out_names), lowering_input_output_aliases=(),
            sim_require_finite=True, sim_require_nnan=True, nc=nc))

    mesh = Mesh(np.asarray(jax.devices()[:M]), ("core",))
    smap = shard_map(_body, mesh=mesh,
                     in_specs=(PartitionSpec("core"),),
                     out_specs=(PartitionSpec("core"),) * len(out_names))
    try:
        fn = bass2jax.fast_dispatch_compile(
            lambda: jax.jit(smap, keep_unused=True).lower(blob).compile())
    except Exception:
        fn = jax.jit(smap, keep_unused=True)
    _CACHE["fn"] = fn
    return fn


def kernel(x, smear_window, knot_mean, knot_low, knot_high,
           ent_mean, ent_low, ent_high, polarization):
    nc = _build()
    blob = _pack_blob(x, smear_window, knot_mean, knot_low, knot_high,
                      ent_mean, ent_low, ent_high, polarization)
    fn = _get_fn(nc, blob)
    out_arrs = fn(blob)
    return np.asarray(out_arrs[0].addressable_shards[0].data)
